# revision 32
# baseline (speedup 1.0000x reference)
"""v4: v3 + exp split across ACT and DVE engines, STT on gpsimd.

Like v3 plus:
- ~half the exp windows run on the vector engine via the Schraudolph
  bit trick: bits16(bf16(exp(2x))) ~= x*256/ln2 + 16250.15, computed by
  one fused tensor_scalar (mult+add, fp32 PSUM -> uint16 view of the
  bf16 ex tile); a second 4x-mode tensor_scalar accumulates the row sum
- the positives STT mask work moves to the (idle) gpsimd engine
- tail copies moved off the scalar engine
"""

import sys

if "/opt/trn_rl_repo" not in sys.path:
    sys.path.insert(0, "/opt/trn_rl_repo")

import numpy as np
import ml_dtypes

N = 16384
D = 128
NC = 8
RPC = N // NC
QB = RPC // 128
NBLK = N // 128
NJJ = 80
TEMP = 0.5
BF16 = ml_dtypes.bfloat16

# Schraudolph bf16 bit-trick exp(2x): bits = x*256/ln2 + C
SMULT = 256.0 / 0.6931471805599453
CADD = 16250.15
DVE_FRAC_NUM, DVE_FRAC_DEN = 2, 7  # fraction of exp windows on DVE
STT_ON_GPSIMD = False  # TRN2 ISA: STT not supported on the Pool/gpsimd engine

_prog_cache = {}


def _seg512(a, b):
    """Split [a, b) at absolute multiples of 512."""
    out = []
    while a < b:
        n = min((a // 512 + 1) * 512, b) - a
        out.append((a, a + n))
        a += n
    return out


def _build_program(dmax):
    import concourse.bacc as bacc
    import concourse.tile as tile
    import concourse.mybir as mybir

    dt = mybir.dt
    AF = mybir.ActivationFunctionType
    ALU = mybir.AluOpType
    AX = mybir.AxisListType

    PW = min(NJJ - 1, QB + dmax)

    nc = bacc.Bacc(
        "TRN2",
        target_bir_lowering=False,
        debug=False,
        enable_asserts=False,
        num_devices=NC,
    )

    kt_d = nc.dram_tensor("kt", [D, N], dt.bfloat16, kind="ExternalInput").ap()
    koh_d = nc.dram_tensor("koh", [128, NJJ * 64], dt.bfloat16, kind="ExternalInput").ap()
    klab_d = nc.dram_tensor("klab", [128, NJJ], dt.float32, kind="ExternalInput").ap()
    qlabb_d = nc.dram_tensor("qlabb", [128, RPC], dt.bfloat16, kind="ExternalInput").ap()
    qoh2_d = nc.dram_tensor("qoh2", [128, RPC], dt.float32, kind="ExternalInput").ap()

    rowacc_d = nc.dram_tensor("rowacc", [128, NJJ - 1], dt.float32, kind="ExternalOutput").ap()
    poskey_d = nc.dram_tensor("poskey", [128, PW], dt.float32, kind="ExternalOutput").ap()
    aq_d = nc.dram_tensor("aq", [1, RPC], dt.float32, kind="ExternalOutput").ap()
    pq_d = nc.dram_tensor("pq", [1, RPC], dt.float32, kind="ExternalOutput").ap()

    with tile.TileContext(nc) as tc:
        with (
            tc.tile_pool(name="keys", bufs=1) as keys_pool,
            tc.tile_pool(name="aux", bufs=1) as aux_pool,
            tc.tile_pool(name="ps", bufs=2, space="PSUM") as psum_pool,
            tc.tile_pool(name="sacc", bufs=1, space="PSUM") as sacc_pool,
            tc.tile_pool(name="ex", bufs=10) as exp_pool,
            tc.tile_pool(name="mk", bufs=5) as msk_pool,
            tc.tile_pool(name="ac", bufs=8) as acc_pool,
            tc.tile_pool(name="fin", bufs=1) as fin_pool,
        ):
            # queries (first 2048 cols) as 2 x 1024 tiles: lets sim
            # matmuls stream 1024 bf16 cols in one instruction
            kt0s = []
            for j in range(2):
                t0 = keys_pool.tile([D, 1024], dt.bfloat16, tag=f"kq{j}", name=f"kq{j}")
                nc.sync.dma_start(t0[:], kt_d[:, j * 1024:(j + 1) * 1024])
                kt0s.append(t0)
            kts = [None]
            for j in range(1, 8):
                ktile = keys_pool.tile([D, 2048], dt.bfloat16, tag=f"kt{j}", name=f"kt{j}")
                nc.sync.dma_start(ktile[:], kt_d[:, j * 2048:(j + 1) * 2048])
                kts.append(ktile)

            def kt_block(b):  # [128, 128] lhsT slice for key block b
                col = b * 128
                if col < 2048:
                    return kt0s[col // 1024][:, col % 1024:col % 1024 + 128]
                return kts[col // 2048][:, col % 2048:col % 2048 + 128]

            def kq_seg(a, b):  # rhs query slice, must sit in one 1024-tile
                assert a // 1024 == (b - 1) // 1024
                return kt0s[a // 1024][:, a % 1024:a % 1024 + (b - a)]

            koh_s = aux_pool.tile([128, NJJ * 64], dt.bfloat16, tag="koh")
            nc.sync.dma_start(koh_s[:], koh_d[:])
            klab_s = aux_pool.tile([128, NJJ], dt.float32, tag="klab")
            nc.sync.dma_start(klab_s[:], klab_d[:])
            qlabb_s = aux_pool.tile([128, RPC], dt.bfloat16, tag="qlabb")
            nc.sync.dma_start(qlabb_s[:], qlabb_d[:])
            qoh2_s = aux_pool.tile([128, RPC], dt.float32, tag="qoh2")
            nc.sync.dma_start(qoh2_s[:], qoh2_d[:])

            zoh = aux_pool.tile([128, 128], dt.bfloat16, tag="zoh")
            nc.vector.memset(zoh[:], 0.0)
            ones128 = aux_pool.tile([128, 1], dt.float32, tag="ones128")
            nc.vector.memset(ones128[:], 1.0)
            dummy = aux_pool.tile([128, 1024], dt.bfloat16, tag="dummy")

            rowacc = fin_pool.tile([128, NJJ - 1], dt.float32, tag="rowacc")
            poskey = fin_pool.tile([128, PW], dt.float32, tag="poskey")

            # doubled S accumulator: rows 0-63 and 64-127 are independent
            # halves (col-group tiling); folded by the final ones-matmul.
            # Zero-init is emitted after do_jj(0) (which has no one-hot
            # matmuls) so the first sim matmuls don't wait on the memset.
            S = sacc_pool.tile([128, RPC], dt.float32, tag="sacc")

            def init_S():
                for t in range(RPC // 512):
                    nc.tensor.matmul(
                        S[:, t * 512:(t + 1) * 512], zoh[:],
                        kt0s[t // 2][:, (t % 2) * 512:(t % 2) * 512 + 512],
                        start=True, stop=False, skip_group_check=True,
                    )

            oh_ctr = [0]

            def oh_mm(jj, a, b, stop, ex, w0):
                for (s0, s1) in _seg512(a, b):
                    th = oh_ctr[0] % 2
                    oh_ctr[0] += 1
                    nc.tensor.matmul(
                        S[th * 64:(th + 1) * 64, s0:s1],
                        koh_s[:, jj * 64:(jj + 1) * 64],
                        ex[:, s0 - w0:s1 - w0],
                        start=False, stop=stop,
                        skip_group_check=True,
                        tile_position=(0, th * 64),
                    )

            wctr = [0]
            stt_eng = nc.gpsimd if STT_ON_GPSIMD else nc.vector

            # deferred-emission queue: dependent work (oh matmuls, STT,
            # reduces) is emitted one window late so no engine queue head
            # ever waits on the exp of the window just issued
            pend = []

            def flush_pend(keep):
                while len(pend) > keep:
                    for fn in pend.pop(0):
                        fn()

            def do_jj(jj):
                main_lo = max(0, jj - 63)
                hi = min(QB - 1, jj)
                lo = (jj - 64 if jj >= 64 else main_lo) * 128
                end = (hi + 1) * 128
                acc_lo = main_lo * 128
                oh_end = min(end, jj * 128) if jj <= QB - 1 else end
                stt_lo = max(acc_lo, (jj - dmax) * 128) if jj <= QB - 1 + dmax else None

                racc = acc_pool.tile([128, 2], dt.float32, tag="racc", name="racc")
                pacc = None
                if stt_lo is not None and stt_lo < end:
                    pacc = acc_pool.tile([128, 2], dt.float32, tag="pacc", name="pacc")
                acc_k = 0
                stt_k = 0
                w0 = (lo // 1024) * 1024
                while w0 < end:
                    p_lo = max(w0, lo)
                    p_end = min(w0 + 1024, end)
                    if p_lo < p_end:
                        ps = psum_pool.tile([128, 1024], dt.float32, tag="ps", name="ps")
                        for (s0, s1) in _seg512(p_lo, p_end):
                            nc.tensor.matmul(
                                ps[:, s0 - w0:s1 - w0],
                                kt_block(jj), kq_seg(s0, s1),
                            )
                        ex = exp_pool.tile([128, 1024], dt.bfloat16, tag="ex", name="ex")
                        # extra (d=64) part: exp only, no accum
                        has_extra = p_lo < acc_lo
                        if has_extra:
                            nc.scalar.activation(
                                ex[:, p_lo - w0:acc_lo - w0],
                                ps[:, p_lo - w0:acc_lo - w0],
                                AF.Exp, scale=1.0 / TEMP,
                            )
                        a0 = max(p_lo, acc_lo)
                        if a0 < p_end:
                            use_dve = (not has_extra) and (
                                (wctr[0] * DVE_FRAC_NUM) // DVE_FRAC_DEN
                                != ((wctr[0] + 1) * DVE_FRAC_NUM) // DVE_FRAC_DEN)
                            wctr[0] += 1
                            kk = acc_k
                            if use_dve:
                                exu = ex[:, a0 - w0:p_end - w0].bitcast(dt.uint16)
                                nc.vector.tensor_scalar(
                                    exu, ps[:, a0 - w0:p_end - w0],
                                    SMULT, CADD, ALU.mult, ALU.add)
                                nc.vector.tensor_scalar(
                                    dummy[:, :p_end - a0],
                                    ex[:, a0 - w0:p_end - w0],
                                    1.0, 0.0, ALU.mult, ALU.add,
                                    accum_out=racc[:, kk:kk + 1])
                            else:
                                nc.scalar.activation(
                                    ex[:, a0 - w0:p_end - w0],
                                    ps[:, a0 - w0:p_end - w0],
                                    AF.Exp, scale=1.0 / TEMP,
                                    accum_out=racc[:, acc_k:acc_k + 1],
                                )
                            acc_k += 1
                        # deferred: one-hot class sums + positives STT for
                        # this window, emitted during the next window
                        todo = []
                        if p_lo < acc_lo:
                            todo.append(lambda jj=jj, p_lo=p_lo, acc_lo=acc_lo,
                                        ex=ex, w0=w0:
                                        oh_mm(jj, p_lo, acc_lo, True, ex, w0))
                        if a0 < min(p_end, oh_end):
                            todo.append(lambda jj=jj, a0=a0,
                                        e=min(p_end, oh_end), ex=ex, w0=w0:
                                        oh_mm(jj, a0, e, False, ex, w0))
                        if pacc is not None:
                            o0 = max(p_lo, stt_lo)
                            if o0 < p_end:
                                kk = stt_k

                                def stt(jj=jj, o0=o0, p_end=p_end, ex=ex,
                                        w0=w0, pacc=pacc, kk=kk):
                                    mk = msk_pool.tile(
                                        [128, 1024], dt.bfloat16, tag="mk", name="mk")
                                    stt_eng.scalar_tensor_tensor(
                                        mk[:, :p_end - o0], qlabb_s[:, o0:p_end],
                                        klab_s[:, jj:jj + 1],
                                        ex[:, o0 - w0:p_end - w0],
                                        ALU.is_equal, ALU.mult,
                                        accum_out=pacc[:, kk:kk + 1],
                                    )
                                todo.append(stt)
                                stt_k += 1
                        pend.append(todo)
                        flush_pend(8)
                    w0 += 1024
                fin = []
                if acc_k:
                    fin.append(lambda jj=jj, racc=racc, n=acc_k:
                               nc.vector.tensor_reduce(
                                   rowacc[:, jj:jj + 1], racc[:, :n],
                                   AX.X, ALU.add))
                if pacc is not None and stt_k:
                    fin.append(lambda jj=jj, pacc=pacc, n=stt_k:
                               nc.vector.tensor_reduce(
                                   poskey[:, jj:jj + 1], pacc[:, :n],
                                   AX.X, ALU.add))
                if fin:
                    pend.append(fin)

            sc = fin_pool.tile([128, RPC], dt.float32, tag="sc")
            pm = fin_pool.tile([128, RPC], dt.float32, tag="pm")

            do_jj(0)
            init_S()
            for jj in range(1, NJJ):
                do_jj(jj)
                # S bank b (query cols [b*512,(b+1)*512)) is final after
                # jj = 4b+3+64: drain + pos-mask multiply early, hidden
                # under the remaining PE work
                if jj >= 67 and (jj - 67) % 4 == 0:
                    b = (jj - 67) // 4

                    def drain(b=b):
                        nc.scalar.copy(
                            sc[:, b * 512:(b + 1) * 512],
                            S[:, b * 512:(b + 1) * 512])
                        nc.vector.tensor_mul(
                            pm[:, b * 512:(b + 1) * 512],
                            sc[:, b * 512:(b + 1) * 512],
                            qoh2_s[:, b * 512:(b + 1) * 512])
                    pend.append([drain])
            flush_pend(0)

            # tail: fold halves via ones-matmuls
            paq = sacc_pool.tile([1, RPC], dt.float32, tag="sacc", name="paq")
            for t in range(RPC // 512):
                nc.tensor.matmul(
                    paq[:, t * 512:(t + 1) * 512], ones128[:],
                    sc[:, t * 512:(t + 1) * 512],
                )
            aq_s = fin_pool.tile([1, RPC], dt.float32, tag="aqs")
            nc.vector.tensor_copy(aq_s[:], paq[:])
            nc.sync.dma_start(aq_d[:], aq_s[:])

            pq_s = fin_pool.tile([1, RPC], dt.float32, tag="pqs")
            for h in range(2):
                ppq = psum_pool.tile([1, 1024], dt.float32, tag="ps", name="ppq")
                for t in range(2):
                    nc.tensor.matmul(
                        ppq[:, t * 512:(t + 1) * 512], ones128[:],
                        pm[:, h * 1024 + t * 512:h * 1024 + (t + 1) * 512],
                    )
                nc.vector.tensor_copy(pq_s[:, h * 1024:(h + 1) * 1024], ppq[:])
            nc.sync.dma_start(pq_d[:], pq_s[:])

            nc.sync.dma_start(rowacc_d[:], rowacc[:])
            nc.sync.dma_start(poskey_d[:], poskey[:])

    nc.compile()
    return nc, PW


def _compute_dmax(lab_s):
    first = lab_s.reshape(NBLK, 128)[:, 0]
    last = lab_s.reshape(NBLK, 128)[:, -1]
    dmax = 0
    for jj in range(NBLK):
        i = jj
        while i > 0 and last[i - 1] >= first[jj]:
            i -= 1
        dmax = max(dmax, jj - i)
    return max(1, min(dmax, 63))


def get_program(dmax):
    key = ("v3", dmax)
    if key not in _prog_cache:
        _prog_cache[key] = _build_program(dmax)
    return _prog_cache[key]


def make_in_maps(embeddings, partition_labels):
    emb = np.asarray(embeddings, dtype=np.float32)
    labels = np.asarray(partition_labels).astype(np.int64)
    perm = np.argsort(labels, kind="stable")
    E_s = emb[perm]
    lab_s = labels[perm]
    lab_f = lab_s.astype(np.float32)

    dmax = _compute_dmax(lab_s)
    E_sT = np.ascontiguousarray(E_s.T).astype(BF16)
    dia = np.exp(np.sum(E_s.astype(np.float64) ** 2, axis=1) / TEMP)

    cls = np.arange(64, dtype=np.int64)
    in_maps = []
    for c in range(NC):
        idx = (np.arange(N) + c * RPC) % N
        ktrot = np.ascontiguousarray(E_sT[:, idx])
        kl = lab_f[idx[:NJJ * 128]].reshape(NJJ, 128).T
        koh = (lab_s[idx[:NJJ * 128]].reshape(NJJ, 128)[:, :, None]
               == cls[None, None, :])
        koh = np.ascontiguousarray(
            koh.transpose(1, 0, 2).reshape(128, NJJ * 64)).astype(BF16)
        qlab_c = lab_f[c * RPC:(c + 1) * RPC]
        qlabb = np.ascontiguousarray(
            np.broadcast_to(qlab_c.astype(BF16)[None, :], (128, RPC)))
        qoh = (lab_s[c * RPC:(c + 1) * RPC][None, :] == cls[:, None])
        qoh2 = np.ascontiguousarray(
            np.vstack([qoh, qoh])).astype(np.float32)      # [128, RPC]
        in_maps.append({
            "kt": ktrot,
            "koh": koh,
            "klab": np.ascontiguousarray(kl).astype(np.float32),
            "qlabb": qlabb,
            "qoh2": qoh2,
        })
    return in_maps, lab_s, dmax, dia


def combine(results, lab_s, PW, dia):
    A = np.zeros(N, dtype=np.float64)
    P = np.zeros(N, dtype=np.float64)
    for c, r in enumerate(results):
        base = c * RPC
        A[base:base + RPC] += np.asarray(r["aq"], dtype=np.float64)[0]
        P[base:base + RPC] += np.asarray(r["pq"], dtype=np.float64)[0]
        ra = np.asarray(r["rowacc"], dtype=np.float64)
        pk = np.asarray(r["poskey"], dtype=np.float64)
        for jj in range(NJJ - 1):
            g = (base + jj * 128) % N
            A[g:g + 128] += ra[:, jj]
            if jj < PW:
                P[g:g + 128] += pk[:, jj]
    A -= dia
    P -= dia

    counts = np.bincount(lab_s, minlength=1)
    valid = counts[lab_s] >= 2
    n_valid = int(valid.sum())
    if n_valid == 0:
        return np.float32(0.0)
    loss = np.log(A) - np.log(np.maximum(P, 1e-300))
    return np.float32(loss[valid].sum() / n_valid)


def kernel(embeddings, partition_labels):
    from concourse.bass_utils import run_bass_kernel_spmd

    in_maps, lab_s, dmax, dia = make_in_maps(embeddings, partition_labels)
    nc, PW = get_program(dmax)
    res = run_bass_kernel_spmd(nc, in_maps, list(range(NC)))
    return combine(res.results, lab_s, PW, dia)



# revision 33
# speedup vs baseline: 1.0261x; 1.0261x over previous
"""v4: v3 + exp split across ACT and DVE engines, STT on gpsimd.

Like v3 plus:
- ~half the exp windows run on the vector engine via the Schraudolph
  bit trick: bits16(bf16(exp(2x))) ~= x*256/ln2 + 16250.15, computed by
  one fused tensor_scalar (mult+add, fp32 PSUM -> uint16 view of the
  bf16 ex tile); a second 4x-mode tensor_scalar accumulates the row sum
- the positives STT mask work moves to the (idle) gpsimd engine
- tail copies moved off the scalar engine
"""

import sys

if "/opt/trn_rl_repo" not in sys.path:
    sys.path.insert(0, "/opt/trn_rl_repo")

import numpy as np
import ml_dtypes

N = 16384
D = 128
NC = 8
RPC = N // NC
QB = RPC // 128
NBLK = N // 128
NJJ = 80
TEMP = 0.5
BF16 = ml_dtypes.bfloat16

# Schraudolph bf16 bit-trick exp(2x): bits = x*256/ln2 + C
SMULT = 256.0 / 0.6931471805599453
CADD = 16250.15
DVE_FRAC_NUM, DVE_FRAC_DEN = 2, 7  # fraction of exp windows on DVE
STT_ON_GPSIMD = False  # TRN2 ISA: STT not supported on the Pool/gpsimd engine

_prog_cache = {}


def _seg512(a, b):
    """Split [a, b) at absolute multiples of 512."""
    out = []
    while a < b:
        n = min((a // 512 + 1) * 512, b) - a
        out.append((a, a + n))
        a += n
    return out


def _build_program(dmax):
    import concourse.bacc as bacc
    import concourse.tile as tile
    import concourse.mybir as mybir

    dt = mybir.dt
    AF = mybir.ActivationFunctionType
    ALU = mybir.AluOpType
    AX = mybir.AxisListType

    PW = min(NJJ - 1, QB + dmax)

    nc = bacc.Bacc(
        "TRN2",
        target_bir_lowering=False,
        debug=False,
        enable_asserts=False,
        num_devices=NC,
    )

    kt_d = nc.dram_tensor("kt", [D, N], dt.bfloat16, kind="ExternalInput").ap()
    koh_d = nc.dram_tensor("koh", [128, NJJ * 64], dt.bfloat16, kind="ExternalInput").ap()
    klab_d = nc.dram_tensor("klab", [128, NJJ], dt.float32, kind="ExternalInput").ap()
    qlabb_d = nc.dram_tensor("qlabb", [128, RPC], dt.bfloat16, kind="ExternalInput").ap()
    qoh2_d = nc.dram_tensor("qoh2", [128, RPC], dt.float32, kind="ExternalInput").ap()

    rowacc_d = nc.dram_tensor("rowacc", [128, NJJ - 1], dt.float32, kind="ExternalOutput").ap()
    poskey_d = nc.dram_tensor("poskey", [128, PW], dt.float32, kind="ExternalOutput").ap()
    aq_d = nc.dram_tensor("aq", [1, RPC], dt.float32, kind="ExternalOutput").ap()
    pq_d = nc.dram_tensor("pq", [1, RPC], dt.float32, kind="ExternalOutput").ap()

    with tile.TileContext(nc) as tc:
        with (
            tc.tile_pool(name="keys", bufs=1) as keys_pool,
            tc.tile_pool(name="aux", bufs=1) as aux_pool,
            tc.tile_pool(name="ps", bufs=2, space="PSUM") as psum_pool,
            tc.tile_pool(name="sacc", bufs=1, space="PSUM") as sacc_pool,
            tc.tile_pool(name="ex", bufs=10) as exp_pool,
            tc.tile_pool(name="mk", bufs=5) as msk_pool,
            tc.tile_pool(name="ac", bufs=8) as acc_pool,
            tc.tile_pool(name="fin", bufs=1) as fin_pool,
        ):
            # queries (first 2048 cols) as 2 x 1024 tiles: lets sim
            # matmuls stream 1024 bf16 cols in one instruction
            kt0s = []
            for j in range(2):
                t0 = keys_pool.tile([D, 1024], dt.bfloat16, tag=f"kq{j}", name=f"kq{j}")
                nc.sync.dma_start(t0[:], kt_d[:, j * 1024:(j + 1) * 1024])
                kt0s.append(t0)
            kts = [None]
            for j in range(1, 8):
                ktile = keys_pool.tile([D, 2048], dt.bfloat16, tag=f"kt{j}", name=f"kt{j}")
                nc.sync.dma_start(ktile[:], kt_d[:, j * 2048:(j + 1) * 2048])
                kts.append(ktile)

            def kt_block(b):  # [128, 128] lhsT slice for key block b
                col = b * 128
                if col < 2048:
                    return kt0s[col // 1024][:, col % 1024:col % 1024 + 128]
                return kts[col // 2048][:, col % 2048:col % 2048 + 128]

            def kq_seg(a, b):  # rhs query slice, must sit in one 1024-tile
                assert a // 1024 == (b - 1) // 1024
                return kt0s[a // 1024][:, a % 1024:a % 1024 + (b - a)]

            koh_s = aux_pool.tile([128, NJJ * 64], dt.bfloat16, tag="koh")
            nc.sync.dma_start(koh_s[:], koh_d[:])
            klab_s = aux_pool.tile([128, NJJ], dt.float32, tag="klab")
            nc.sync.dma_start(klab_s[:], klab_d[:])
            qlabb_s = aux_pool.tile([128, RPC], dt.bfloat16, tag="qlabb")
            nc.sync.dma_start(qlabb_s[:], qlabb_d[:])
            qoh2_s = aux_pool.tile([128, RPC], dt.float32, tag="qoh2")
            nc.sync.dma_start(qoh2_s[:], qoh2_d[:])

            zoh = aux_pool.tile([128, 128], dt.bfloat16, tag="zoh")
            nc.vector.memset(zoh[:], 0.0)
            ones128 = aux_pool.tile([128, 1], dt.float32, tag="ones128")
            nc.vector.memset(ones128[:], 1.0)
            dummy = aux_pool.tile([128, 1024], dt.bfloat16, tag="dummy")

            rowacc = fin_pool.tile([128, NJJ - 1], dt.float32, tag="rowacc")
            poskey = fin_pool.tile([128, PW], dt.float32, tag="poskey")

            # doubled S accumulator: rows 0-63 and 64-127 are independent
            # halves (col-group tiling); folded by the final ones-matmul.
            # Zero-init is emitted after do_jj(0) (which has no one-hot
            # matmuls) so the first sim matmuls don't wait on the memset.
            S = sacc_pool.tile([128, RPC], dt.float32, tag="sacc")

            def init_S():
                for t in range(RPC // 512):
                    nc.tensor.matmul(
                        S[:, t * 512:(t + 1) * 512], zoh[:],
                        kt0s[t // 2][:, (t % 2) * 512:(t % 2) * 512 + 512],
                        start=True, stop=False, skip_group_check=True,
                    )

            oh_ctr = [0]

            def oh_mm(jj, a, b, stop, ex, w0):
                for (s0, s1) in _seg512(a, b):
                    th = oh_ctr[0] % 2
                    oh_ctr[0] += 1
                    nc.tensor.matmul(
                        S[th * 64:(th + 1) * 64, s0:s1],
                        koh_s[:, jj * 64:(jj + 1) * 64],
                        ex[:, s0 - w0:s1 - w0],
                        start=False, stop=stop,
                        skip_group_check=True,
                        tile_position=(0, th * 64),
                    )

            wctr = [0]
            stt_eng = nc.gpsimd if STT_ON_GPSIMD else nc.vector

            # deferred-emission queue: dependent work (oh matmuls, STT,
            # reduces) is emitted one window late so no engine queue head
            # ever waits on the exp of the window just issued
            pend = []

            def flush_pend(keep):
                while len(pend) > keep:
                    for fn in pend.pop(0):
                        fn()

            def do_jj(jj):
                main_lo = max(0, jj - 63)
                hi = min(QB - 1, jj)
                lo = (jj - 64 if jj >= 64 else main_lo) * 128
                end = (hi + 1) * 128
                acc_lo = main_lo * 128
                oh_end = min(end, jj * 128) if jj <= QB - 1 else end
                stt_lo = max(acc_lo, (jj - dmax) * 128) if jj <= QB - 1 + dmax else None

                racc = acc_pool.tile([128, 2], dt.float32, tag="racc", name="racc")
                pacc = None
                if stt_lo is not None and stt_lo < end:
                    pacc = acc_pool.tile([128, 2], dt.float32, tag="pacc", name="pacc")
                acc_k = 0
                stt_k = 0
                w0 = (lo // 1024) * 1024
                while w0 < end:
                    p_lo = max(w0, lo)
                    p_end = min(w0 + 1024, end)
                    if p_lo < p_end:
                        ps = psum_pool.tile([128, 1024], dt.float32, tag="ps", name="ps")
                        for (s0, s1) in _seg512(p_lo, p_end):
                            nc.tensor.matmul(
                                ps[:, s0 - w0:s1 - w0],
                                kt_block(jj), kq_seg(s0, s1),
                            )
                        ex = exp_pool.tile([128, 1024], dt.bfloat16, tag="ex", name="ex")
                        # extra (d=64) part: exp only, no accum
                        has_extra = p_lo < acc_lo
                        if has_extra:
                            nc.scalar.activation(
                                ex[:, p_lo - w0:acc_lo - w0],
                                ps[:, p_lo - w0:acc_lo - w0],
                                AF.Exp, scale=1.0 / TEMP,
                            )
                        a0 = max(p_lo, acc_lo)
                        if a0 < p_end:
                            use_dve = (not has_extra) and (
                                (wctr[0] * DVE_FRAC_NUM) // DVE_FRAC_DEN
                                != ((wctr[0] + 1) * DVE_FRAC_NUM) // DVE_FRAC_DEN)
                            wctr[0] += 1
                            kk = acc_k
                            if use_dve:
                                exu = ex[:, a0 - w0:p_end - w0].bitcast(dt.uint16)
                                nc.vector.tensor_scalar(
                                    exu, ps[:, a0 - w0:p_end - w0],
                                    SMULT, CADD, ALU.mult, ALU.add)
                                nc.vector.tensor_scalar(
                                    dummy[:, :p_end - a0],
                                    ex[:, a0 - w0:p_end - w0],
                                    1.0, 0.0, ALU.mult, ALU.add,
                                    accum_out=racc[:, kk:kk + 1])
                            else:
                                nc.scalar.activation(
                                    ex[:, a0 - w0:p_end - w0],
                                    ps[:, a0 - w0:p_end - w0],
                                    AF.Exp, scale=1.0 / TEMP,
                                    accum_out=racc[:, acc_k:acc_k + 1],
                                )
                            acc_k += 1
                        # deferred: one-hot class sums + positives STT for
                        # this window, emitted during the next window
                        todo = []
                        if p_lo < acc_lo:
                            todo.append(lambda jj=jj, p_lo=p_lo, acc_lo=acc_lo,
                                        ex=ex, w0=w0:
                                        oh_mm(jj, p_lo, acc_lo, True, ex, w0))
                        if a0 < min(p_end, oh_end):
                            todo.append(lambda jj=jj, a0=a0,
                                        e=min(p_end, oh_end), ex=ex, w0=w0:
                                        oh_mm(jj, a0, e, False, ex, w0))
                        if pacc is not None:
                            o0 = max(p_lo, stt_lo)
                            if o0 < p_end:
                                kk = stt_k

                                def stt(jj=jj, o0=o0, p_end=p_end, ex=ex,
                                        w0=w0, pacc=pacc, kk=kk):
                                    mk = msk_pool.tile(
                                        [128, 1024], dt.bfloat16, tag="mk", name="mk")
                                    stt_eng.scalar_tensor_tensor(
                                        mk[:, :p_end - o0], qlabb_s[:, o0:p_end],
                                        klab_s[:, jj:jj + 1],
                                        ex[:, o0 - w0:p_end - w0],
                                        ALU.is_equal, ALU.mult,
                                        accum_out=pacc[:, kk:kk + 1],
                                    )
                                todo.append(stt)
                                stt_k += 1
                        pend.append(todo)
                        flush_pend(8)
                    w0 += 1024
                fin = []
                if acc_k:
                    fin.append(lambda jj=jj, racc=racc, n=acc_k:
                               nc.vector.tensor_reduce(
                                   rowacc[:, jj:jj + 1], racc[:, :n],
                                   AX.X, ALU.add))
                if pacc is not None and stt_k:
                    fin.append(lambda jj=jj, pacc=pacc, n=stt_k:
                               nc.vector.tensor_reduce(
                                   poskey[:, jj:jj + 1], pacc[:, :n],
                                   AX.X, ALU.add))
                if fin:
                    pend.append(fin)

            sc = fin_pool.tile([128, RPC], dt.float32, tag="sc")
            pm = fin_pool.tile([128, RPC], dt.float32, tag="pm")

            do_jj(0)
            init_S()
            # interleave small head/tail jj's among full-width ones so the
            # pipeline never runs a long stretch of short windows
            fulls = list(range(16, 64))
            others = list(range(1, 16)) + list(range(64, NJJ))
            seq = []
            fi = oi = 0
            while fi < len(fulls) or oi < len(others):
                if oi >= len(others) or (
                        fi < len(fulls)
                        and fi * len(others) <= oi * len(fulls)):
                    seq.append(fulls[fi]); fi += 1
                else:
                    seq.append(others[oi]); oi += 1

            def oh_iv(jj):  # cols covered by jj's oh matmuls
                main_lo = max(0, jj - 63)
                hi = min(QB - 1, jj)
                lo = (jj - 64 if jj >= 64 else main_lo) * 128
                end = (hi + 1) * 128
                oh_end = min(end, jj * 128) if jj <= QB - 1 else end
                return lo, oh_end

            last_toucher = {}
            for b in range(4):
                c0, c1 = b * 512, (b + 1) * 512
                for pos, jj in enumerate(seq):
                    lo, oh_end = oh_iv(jj)
                    if lo < c1 and oh_end > c0:
                        last_toucher[b] = pos

            def drain(b):
                nc.scalar.copy(
                    sc[:, b * 512:(b + 1) * 512],
                    S[:, b * 512:(b + 1) * 512])
                nc.vector.tensor_mul(
                    pm[:, b * 512:(b + 1) * 512],
                    sc[:, b * 512:(b + 1) * 512],
                    qoh2_s[:, b * 512:(b + 1) * 512])

            for pos, jj in enumerate(seq):
                do_jj(jj)
                for b in range(4):
                    if last_toucher.get(b) == pos:
                        pend.append([lambda b=b: drain(b)])
            flush_pend(0)

            # tail: fold halves via ones-matmuls
            paq = sacc_pool.tile([1, RPC], dt.float32, tag="sacc", name="paq")
            for t in range(RPC // 512):
                nc.tensor.matmul(
                    paq[:, t * 512:(t + 1) * 512], ones128[:],
                    sc[:, t * 512:(t + 1) * 512],
                )
            aq_s = fin_pool.tile([1, RPC], dt.float32, tag="aqs")
            nc.vector.tensor_copy(aq_s[:], paq[:])
            nc.sync.dma_start(aq_d[:], aq_s[:])

            pq_s = fin_pool.tile([1, RPC], dt.float32, tag="pqs")
            for h in range(2):
                ppq = psum_pool.tile([1, 1024], dt.float32, tag="ps", name="ppq")
                for t in range(2):
                    nc.tensor.matmul(
                        ppq[:, t * 512:(t + 1) * 512], ones128[:],
                        pm[:, h * 1024 + t * 512:h * 1024 + (t + 1) * 512],
                    )
                nc.vector.tensor_copy(pq_s[:, h * 1024:(h + 1) * 1024], ppq[:])
            nc.sync.dma_start(pq_d[:], pq_s[:])

            nc.sync.dma_start(rowacc_d[:], rowacc[:])
            nc.sync.dma_start(poskey_d[:], poskey[:])

    nc.compile()
    return nc, PW


def _compute_dmax(lab_s):
    first = lab_s.reshape(NBLK, 128)[:, 0]
    last = lab_s.reshape(NBLK, 128)[:, -1]
    dmax = 0
    for jj in range(NBLK):
        i = jj
        while i > 0 and last[i - 1] >= first[jj]:
            i -= 1
        dmax = max(dmax, jj - i)
    return max(1, min(dmax, 63))


def get_program(dmax):
    key = ("v3", dmax)
    if key not in _prog_cache:
        _prog_cache[key] = _build_program(dmax)
    return _prog_cache[key]


def make_in_maps(embeddings, partition_labels):
    emb = np.asarray(embeddings, dtype=np.float32)
    labels = np.asarray(partition_labels).astype(np.int64)
    perm = np.argsort(labels, kind="stable")
    E_s = emb[perm]
    lab_s = labels[perm]
    lab_f = lab_s.astype(np.float32)

    dmax = _compute_dmax(lab_s)
    E_sT = np.ascontiguousarray(E_s.T).astype(BF16)
    dia = np.exp(np.sum(E_s.astype(np.float64) ** 2, axis=1) / TEMP)

    cls = np.arange(64, dtype=np.int64)
    in_maps = []
    for c in range(NC):
        idx = (np.arange(N) + c * RPC) % N
        ktrot = np.ascontiguousarray(E_sT[:, idx])
        kl = lab_f[idx[:NJJ * 128]].reshape(NJJ, 128).T
        koh = (lab_s[idx[:NJJ * 128]].reshape(NJJ, 128)[:, :, None]
               == cls[None, None, :])
        koh = np.ascontiguousarray(
            koh.transpose(1, 0, 2).reshape(128, NJJ * 64)).astype(BF16)
        qlab_c = lab_f[c * RPC:(c + 1) * RPC]
        qlabb = np.ascontiguousarray(
            np.broadcast_to(qlab_c.astype(BF16)[None, :], (128, RPC)))
        qoh = (lab_s[c * RPC:(c + 1) * RPC][None, :] == cls[:, None])
        qoh2 = np.ascontiguousarray(
            np.vstack([qoh, qoh])).astype(np.float32)      # [128, RPC]
        in_maps.append({
            "kt": ktrot,
            "koh": koh,
            "klab": np.ascontiguousarray(kl).astype(np.float32),
            "qlabb": qlabb,
            "qoh2": qoh2,
        })
    return in_maps, lab_s, dmax, dia


def combine(results, lab_s, PW, dia):
    A = np.zeros(N, dtype=np.float64)
    P = np.zeros(N, dtype=np.float64)
    for c, r in enumerate(results):
        base = c * RPC
        A[base:base + RPC] += np.asarray(r["aq"], dtype=np.float64)[0]
        P[base:base + RPC] += np.asarray(r["pq"], dtype=np.float64)[0]
        ra = np.asarray(r["rowacc"], dtype=np.float64)
        pk = np.asarray(r["poskey"], dtype=np.float64)
        for jj in range(NJJ - 1):
            g = (base + jj * 128) % N
            A[g:g + 128] += ra[:, jj]
            if jj < PW:
                P[g:g + 128] += pk[:, jj]
    A -= dia
    P -= dia

    counts = np.bincount(lab_s, minlength=1)
    valid = counts[lab_s] >= 2
    n_valid = int(valid.sum())
    if n_valid == 0:
        return np.float32(0.0)
    loss = np.log(A) - np.log(np.maximum(P, 1e-300))
    return np.float32(loss[valid].sum() / n_valid)


def kernel(embeddings, partition_labels):
    from concourse.bass_utils import run_bass_kernel_spmd

    in_maps, lab_s, dmax, dia = make_in_maps(embeddings, partition_labels)
    nc, PW = get_program(dmax)
    res = run_bass_kernel_spmd(nc, in_maps, list(range(NC)))
    return combine(res.results, lab_s, PW, dia)



# revision 34
# speedup vs baseline: 1.0264x; 1.0002x over previous
"""v4: v3 + exp split across ACT and DVE engines, STT on gpsimd.

Like v3 plus:
- ~half the exp windows run on the vector engine via the Schraudolph
  bit trick: bits16(bf16(exp(2x))) ~= x*256/ln2 + 16250.15, computed by
  one fused tensor_scalar (mult+add, fp32 PSUM -> uint16 view of the
  bf16 ex tile); a second 4x-mode tensor_scalar accumulates the row sum
- the positives STT mask work moves to the (idle) gpsimd engine
- tail copies moved off the scalar engine
"""

import sys

if "/opt/trn_rl_repo" not in sys.path:
    sys.path.insert(0, "/opt/trn_rl_repo")

import numpy as np
import ml_dtypes

N = 16384
D = 128
NC = 8
RPC = N // NC
QB = RPC // 128
NBLK = N // 128
NJJ = 80
TEMP = 0.5
BF16 = ml_dtypes.bfloat16

# Schraudolph bf16 bit-trick exp(2x): bits = x*256/ln2 + C
SMULT = 256.0 / 0.6931471805599453
CADD = 16250.15
DVE_FRAC_NUM, DVE_FRAC_DEN = 1, 4  # fraction of exp windows on DVE
STT_ON_GPSIMD = False  # TRN2 ISA: STT not supported on the Pool/gpsimd engine

_prog_cache = {}


def _seg512(a, b):
    """Split [a, b) at absolute multiples of 512."""
    out = []
    while a < b:
        n = min((a // 512 + 1) * 512, b) - a
        out.append((a, a + n))
        a += n
    return out


def _build_program(dmax):
    import concourse.bacc as bacc
    import concourse.tile as tile
    import concourse.mybir as mybir

    dt = mybir.dt
    AF = mybir.ActivationFunctionType
    ALU = mybir.AluOpType
    AX = mybir.AxisListType

    PW = min(NJJ - 1, QB + dmax)

    nc = bacc.Bacc(
        "TRN2",
        target_bir_lowering=False,
        debug=False,
        enable_asserts=False,
        num_devices=NC,
    )

    kt_d = nc.dram_tensor("kt", [D, N], dt.bfloat16, kind="ExternalInput").ap()
    koh_d = nc.dram_tensor("koh", [128, NJJ * 64], dt.bfloat16, kind="ExternalInput").ap()
    klab_d = nc.dram_tensor("klab", [128, NJJ], dt.float32, kind="ExternalInput").ap()
    qlabb_d = nc.dram_tensor("qlabb", [128, RPC], dt.bfloat16, kind="ExternalInput").ap()
    qoh2_d = nc.dram_tensor("qoh2", [128, RPC], dt.float32, kind="ExternalInput").ap()

    rowacc_d = nc.dram_tensor("rowacc", [128, NJJ - 1], dt.float32, kind="ExternalOutput").ap()
    poskey_d = nc.dram_tensor("poskey", [128, PW], dt.float32, kind="ExternalOutput").ap()
    aq_d = nc.dram_tensor("aq", [1, RPC], dt.float32, kind="ExternalOutput").ap()
    pq_d = nc.dram_tensor("pq", [1, RPC], dt.float32, kind="ExternalOutput").ap()

    with tile.TileContext(nc) as tc:
        with (
            tc.tile_pool(name="keys", bufs=1) as keys_pool,
            tc.tile_pool(name="aux", bufs=1) as aux_pool,
            tc.tile_pool(name="ps", bufs=2, space="PSUM") as psum_pool,
            tc.tile_pool(name="sacc", bufs=1, space="PSUM") as sacc_pool,
            tc.tile_pool(name="ex", bufs=10) as exp_pool,
            tc.tile_pool(name="mk", bufs=5) as msk_pool,
            tc.tile_pool(name="ac", bufs=8) as acc_pool,
            tc.tile_pool(name="fin", bufs=1) as fin_pool,
        ):
            # queries (first 2048 cols) as 2 x 1024 tiles: lets sim
            # matmuls stream 1024 bf16 cols in one instruction
            kt0s = []
            for j in range(2):
                t0 = keys_pool.tile([D, 1024], dt.bfloat16, tag=f"kq{j}", name=f"kq{j}")
                nc.sync.dma_start(t0[:], kt_d[:, j * 1024:(j + 1) * 1024])
                kt0s.append(t0)
            kts = [None]
            for j in range(1, 8):
                ktile = keys_pool.tile([D, 2048], dt.bfloat16, tag=f"kt{j}", name=f"kt{j}")
                nc.sync.dma_start(ktile[:], kt_d[:, j * 2048:(j + 1) * 2048])
                kts.append(ktile)

            def kt_block(b):  # [128, 128] lhsT slice for key block b
                col = b * 128
                if col < 2048:
                    return kt0s[col // 1024][:, col % 1024:col % 1024 + 128]
                return kts[col // 2048][:, col % 2048:col % 2048 + 128]

            def kq_seg(a, b):  # rhs query slice, must sit in one 1024-tile
                assert a // 1024 == (b - 1) // 1024
                return kt0s[a // 1024][:, a % 1024:a % 1024 + (b - a)]

            koh_s = aux_pool.tile([128, NJJ * 64], dt.bfloat16, tag="koh")
            nc.sync.dma_start(koh_s[:], koh_d[:])
            klab_s = aux_pool.tile([128, NJJ], dt.float32, tag="klab")
            nc.sync.dma_start(klab_s[:], klab_d[:])
            qlabb_s = aux_pool.tile([128, RPC], dt.bfloat16, tag="qlabb")
            nc.sync.dma_start(qlabb_s[:], qlabb_d[:])
            qoh2_s = aux_pool.tile([128, RPC], dt.float32, tag="qoh2")
            nc.sync.dma_start(qoh2_s[:], qoh2_d[:])

            zoh = aux_pool.tile([128, 128], dt.bfloat16, tag="zoh")
            nc.vector.memset(zoh[:], 0.0)
            ones128 = aux_pool.tile([128, 1], dt.float32, tag="ones128")
            nc.vector.memset(ones128[:], 1.0)
            dummy = aux_pool.tile([128, 1024], dt.bfloat16, tag="dummy")

            rowacc = fin_pool.tile([128, NJJ - 1], dt.float32, tag="rowacc")
            poskey = fin_pool.tile([128, PW], dt.float32, tag="poskey")

            # doubled S accumulator: rows 0-63 and 64-127 are independent
            # halves (col-group tiling); folded by the final ones-matmul.
            # Zero-init is emitted after do_jj(0) (which has no one-hot
            # matmuls) so the first sim matmuls don't wait on the memset.
            S = sacc_pool.tile([128, RPC], dt.float32, tag="sacc")

            def init_S():
                for t in range(RPC // 512):
                    nc.tensor.matmul(
                        S[:, t * 512:(t + 1) * 512], zoh[:],
                        kt0s[t // 2][:, (t % 2) * 512:(t % 2) * 512 + 512],
                        start=True, stop=False, skip_group_check=True,
                    )

            oh_ctr = [0]

            def oh_mm(jj, a, b, stop, ex, w0):
                for (s0, s1) in _seg512(a, b):
                    th = oh_ctr[0] % 2
                    oh_ctr[0] += 1
                    nc.tensor.matmul(
                        S[th * 64:(th + 1) * 64, s0:s1],
                        koh_s[:, jj * 64:(jj + 1) * 64],
                        ex[:, s0 - w0:s1 - w0],
                        start=False, stop=stop,
                        skip_group_check=True,
                        tile_position=(0, th * 64),
                    )

            wctr = [0]
            stt_eng = nc.gpsimd if STT_ON_GPSIMD else nc.vector

            # deferred-emission queue: dependent work (oh matmuls, STT,
            # reduces) is emitted one window late so no engine queue head
            # ever waits on the exp of the window just issued
            pend = []

            def flush_pend(keep):
                while len(pend) > keep:
                    for fn in pend.pop(0):
                        fn()

            def do_jj(jj):
                main_lo = max(0, jj - 63)
                hi = min(QB - 1, jj)
                lo = (jj - 64 if jj >= 64 else main_lo) * 128
                end = (hi + 1) * 128
                acc_lo = main_lo * 128
                oh_end = min(end, jj * 128) if jj <= QB - 1 else end
                stt_lo = max(acc_lo, (jj - dmax) * 128) if jj <= QB - 1 + dmax else None

                racc = acc_pool.tile([128, 2], dt.float32, tag="racc", name="racc")
                pacc = None
                if stt_lo is not None and stt_lo < end:
                    pacc = acc_pool.tile([128, 2], dt.float32, tag="pacc", name="pacc")
                acc_k = 0
                stt_k = 0
                w0 = (lo // 1024) * 1024
                while w0 < end:
                    p_lo = max(w0, lo)
                    p_end = min(w0 + 1024, end)
                    if p_lo < p_end:
                        ps = psum_pool.tile([128, 1024], dt.float32, tag="ps", name="ps")
                        for (s0, s1) in _seg512(p_lo, p_end):
                            nc.tensor.matmul(
                                ps[:, s0 - w0:s1 - w0],
                                kt_block(jj), kq_seg(s0, s1),
                            )
                        ex = exp_pool.tile([128, 1024], dt.bfloat16, tag="ex", name="ex")
                        # extra (d=64) part: exp only, no accum
                        has_extra = p_lo < acc_lo
                        if has_extra:
                            nc.scalar.activation(
                                ex[:, p_lo - w0:acc_lo - w0],
                                ps[:, p_lo - w0:acc_lo - w0],
                                AF.Exp, scale=1.0 / TEMP,
                            )
                        a0 = max(p_lo, acc_lo)
                        if a0 < p_end:
                            use_dve = (not has_extra) and (
                                (wctr[0] * DVE_FRAC_NUM) // DVE_FRAC_DEN
                                != ((wctr[0] + 1) * DVE_FRAC_NUM) // DVE_FRAC_DEN)
                            wctr[0] += 1
                            kk = acc_k
                            if use_dve:
                                exu = ex[:, a0 - w0:p_end - w0].bitcast(dt.uint16)
                                nc.vector.tensor_scalar(
                                    exu, ps[:, a0 - w0:p_end - w0],
                                    SMULT, CADD, ALU.mult, ALU.add)
                                nc.vector.tensor_scalar(
                                    dummy[:, :p_end - a0],
                                    ex[:, a0 - w0:p_end - w0],
                                    1.0, 0.0, ALU.mult, ALU.add,
                                    accum_out=racc[:, kk:kk + 1])
                            else:
                                nc.scalar.activation(
                                    ex[:, a0 - w0:p_end - w0],
                                    ps[:, a0 - w0:p_end - w0],
                                    AF.Exp, scale=1.0 / TEMP,
                                    accum_out=racc[:, acc_k:acc_k + 1],
                                )
                            acc_k += 1
                        # deferred: one-hot class sums + positives STT for
                        # this window, emitted during the next window
                        todo = []
                        if p_lo < acc_lo:
                            todo.append(lambda jj=jj, p_lo=p_lo, acc_lo=acc_lo,
                                        ex=ex, w0=w0:
                                        oh_mm(jj, p_lo, acc_lo, True, ex, w0))
                        if a0 < min(p_end, oh_end):
                            todo.append(lambda jj=jj, a0=a0,
                                        e=min(p_end, oh_end), ex=ex, w0=w0:
                                        oh_mm(jj, a0, e, False, ex, w0))
                        if pacc is not None:
                            o0 = max(p_lo, stt_lo)
                            if o0 < p_end:
                                kk = stt_k

                                def stt(jj=jj, o0=o0, p_end=p_end, ex=ex,
                                        w0=w0, pacc=pacc, kk=kk):
                                    mk = msk_pool.tile(
                                        [128, 1024], dt.bfloat16, tag="mk", name="mk")
                                    stt_eng.scalar_tensor_tensor(
                                        mk[:, :p_end - o0], qlabb_s[:, o0:p_end],
                                        klab_s[:, jj:jj + 1],
                                        ex[:, o0 - w0:p_end - w0],
                                        ALU.is_equal, ALU.mult,
                                        accum_out=pacc[:, kk:kk + 1],
                                    )
                                todo.append(stt)
                                stt_k += 1
                        pend.append(todo)
                        flush_pend(8)
                    w0 += 1024
                fin = []
                if acc_k:
                    fin.append(lambda jj=jj, racc=racc, n=acc_k:
                               nc.vector.tensor_reduce(
                                   rowacc[:, jj:jj + 1], racc[:, :n],
                                   AX.X, ALU.add))
                if pacc is not None and stt_k:
                    fin.append(lambda jj=jj, pacc=pacc, n=stt_k:
                               nc.vector.tensor_reduce(
                                   poskey[:, jj:jj + 1], pacc[:, :n],
                                   AX.X, ALU.add))
                if fin:
                    pend.append(fin)

            sc = fin_pool.tile([128, RPC], dt.float32, tag="sc")
            pm = fin_pool.tile([128, RPC], dt.float32, tag="pm")

            do_jj(0)
            init_S()
            # interleave small head/tail jj's among full-width ones so the
            # pipeline never runs a long stretch of short windows
            fulls = list(range(16, 64))
            others = list(range(1, 16)) + list(range(64, NJJ))
            seq = []
            fi = oi = 0
            while fi < len(fulls) or oi < len(others):
                if oi >= len(others) or (
                        fi < len(fulls)
                        and fi * len(others) <= oi * len(fulls)):
                    seq.append(fulls[fi]); fi += 1
                else:
                    seq.append(others[oi]); oi += 1

            def oh_iv(jj):  # cols covered by jj's oh matmuls
                main_lo = max(0, jj - 63)
                hi = min(QB - 1, jj)
                lo = (jj - 64 if jj >= 64 else main_lo) * 128
                end = (hi + 1) * 128
                oh_end = min(end, jj * 128) if jj <= QB - 1 else end
                return lo, oh_end

            last_toucher = {}
            for b in range(4):
                c0, c1 = b * 512, (b + 1) * 512
                for pos, jj in enumerate(seq):
                    lo, oh_end = oh_iv(jj)
                    if lo < c1 and oh_end > c0:
                        last_toucher[b] = pos

            def drain(b):
                nc.scalar.copy(
                    sc[:, b * 512:(b + 1) * 512],
                    S[:, b * 512:(b + 1) * 512])
                nc.vector.tensor_mul(
                    pm[:, b * 512:(b + 1) * 512],
                    sc[:, b * 512:(b + 1) * 512],
                    qoh2_s[:, b * 512:(b + 1) * 512])

            for pos, jj in enumerate(seq):
                do_jj(jj)
                for b in range(4):
                    if last_toucher.get(b) == pos:
                        pend.append([lambda b=b: drain(b)])
            flush_pend(0)

            # tail: fold halves via ones-matmuls
            paq = sacc_pool.tile([1, RPC], dt.float32, tag="sacc", name="paq")
            for t in range(RPC // 512):
                nc.tensor.matmul(
                    paq[:, t * 512:(t + 1) * 512], ones128[:],
                    sc[:, t * 512:(t + 1) * 512],
                )
            aq_s = fin_pool.tile([1, RPC], dt.float32, tag="aqs")
            nc.vector.tensor_copy(aq_s[:], paq[:])
            nc.sync.dma_start(aq_d[:], aq_s[:])

            pq_s = fin_pool.tile([1, RPC], dt.float32, tag="pqs")
            for h in range(2):
                ppq = psum_pool.tile([1, 1024], dt.float32, tag="ps", name="ppq")
                for t in range(2):
                    nc.tensor.matmul(
                        ppq[:, t * 512:(t + 1) * 512], ones128[:],
                        pm[:, h * 1024 + t * 512:h * 1024 + (t + 1) * 512],
                    )
                nc.vector.tensor_copy(pq_s[:, h * 1024:(h + 1) * 1024], ppq[:])
            nc.sync.dma_start(pq_d[:], pq_s[:])

            nc.sync.dma_start(rowacc_d[:], rowacc[:])
            nc.sync.dma_start(poskey_d[:], poskey[:])

    nc.compile()
    return nc, PW


def _compute_dmax(lab_s):
    first = lab_s.reshape(NBLK, 128)[:, 0]
    last = lab_s.reshape(NBLK, 128)[:, -1]
    dmax = 0
    for jj in range(NBLK):
        i = jj
        while i > 0 and last[i - 1] >= first[jj]:
            i -= 1
        dmax = max(dmax, jj - i)
    return max(1, min(dmax, 63))


def get_program(dmax):
    key = ("v3", dmax)
    if key not in _prog_cache:
        _prog_cache[key] = _build_program(dmax)
    return _prog_cache[key]


def make_in_maps(embeddings, partition_labels):
    emb = np.asarray(embeddings, dtype=np.float32)
    labels = np.asarray(partition_labels).astype(np.int64)
    perm = np.argsort(labels, kind="stable")
    E_s = emb[perm]
    lab_s = labels[perm]
    lab_f = lab_s.astype(np.float32)

    dmax = _compute_dmax(lab_s)
    E_sT = np.ascontiguousarray(E_s.T).astype(BF16)
    dia = np.exp(np.sum(E_s.astype(np.float64) ** 2, axis=1) / TEMP)

    cls = np.arange(64, dtype=np.int64)
    in_maps = []
    for c in range(NC):
        idx = (np.arange(N) + c * RPC) % N
        ktrot = np.ascontiguousarray(E_sT[:, idx])
        kl = lab_f[idx[:NJJ * 128]].reshape(NJJ, 128).T
        koh = (lab_s[idx[:NJJ * 128]].reshape(NJJ, 128)[:, :, None]
               == cls[None, None, :])
        koh = np.ascontiguousarray(
            koh.transpose(1, 0, 2).reshape(128, NJJ * 64)).astype(BF16)
        qlab_c = lab_f[c * RPC:(c + 1) * RPC]
        qlabb = np.ascontiguousarray(
            np.broadcast_to(qlab_c.astype(BF16)[None, :], (128, RPC)))
        qoh = (lab_s[c * RPC:(c + 1) * RPC][None, :] == cls[:, None])
        qoh2 = np.ascontiguousarray(
            np.vstack([qoh, qoh])).astype(np.float32)      # [128, RPC]
        in_maps.append({
            "kt": ktrot,
            "koh": koh,
            "klab": np.ascontiguousarray(kl).astype(np.float32),
            "qlabb": qlabb,
            "qoh2": qoh2,
        })
    return in_maps, lab_s, dmax, dia


def combine(results, lab_s, PW, dia):
    A = np.zeros(N, dtype=np.float64)
    P = np.zeros(N, dtype=np.float64)
    for c, r in enumerate(results):
        base = c * RPC
        A[base:base + RPC] += np.asarray(r["aq"], dtype=np.float64)[0]
        P[base:base + RPC] += np.asarray(r["pq"], dtype=np.float64)[0]
        ra = np.asarray(r["rowacc"], dtype=np.float64)
        pk = np.asarray(r["poskey"], dtype=np.float64)
        for jj in range(NJJ - 1):
            g = (base + jj * 128) % N
            A[g:g + 128] += ra[:, jj]
            if jj < PW:
                P[g:g + 128] += pk[:, jj]
    A -= dia
    P -= dia

    counts = np.bincount(lab_s, minlength=1)
    valid = counts[lab_s] >= 2
    n_valid = int(valid.sum())
    if n_valid == 0:
        return np.float32(0.0)
    loss = np.log(A) - np.log(np.maximum(P, 1e-300))
    return np.float32(loss[valid].sum() / n_valid)


def kernel(embeddings, partition_labels):
    from concourse.bass_utils import run_bass_kernel_spmd

    in_maps, lab_s, dmax, dia = make_in_maps(embeddings, partition_labels)
    nc, PW = get_program(dmax)
    res = run_bass_kernel_spmd(nc, in_maps, list(range(NC)))
    return combine(res.results, lab_s, PW, dia)



# revision 35
# speedup vs baseline: 1.0390x; 1.0122x over previous
"""v4: v3 + exp split across ACT and DVE engines, STT on gpsimd.

Like v3 plus:
- ~half the exp windows run on the vector engine via the Schraudolph
  bit trick: bits16(bf16(exp(2x))) ~= x*256/ln2 + 16250.15, computed by
  one fused tensor_scalar (mult+add, fp32 PSUM -> uint16 view of the
  bf16 ex tile); a second 4x-mode tensor_scalar accumulates the row sum
- the positives STT mask work moves to the (idle) gpsimd engine
- tail copies moved off the scalar engine
"""

import sys

if "/opt/trn_rl_repo" not in sys.path:
    sys.path.insert(0, "/opt/trn_rl_repo")

import numpy as np
import ml_dtypes

N = 16384
D = 128
NC = 8
RPC = N // NC
QB = RPC // 128
NBLK = N // 128
NJJ = 80
TEMP = 0.5
BF16 = ml_dtypes.bfloat16

# Schraudolph bf16 bit-trick exp(2x): bits = x*256/ln2 + C
SMULT = 256.0 / 0.6931471805599453
CADD = 16250.15
DVE_FRAC_NUM, DVE_FRAC_DEN = 2, 7  # fraction of exp windows on DVE
STT_ON_GPSIMD = False  # TRN2 ISA: STT not supported on the Pool/gpsimd engine

_prog_cache = {}


def _seg512(a, b):
    """Split [a, b) at absolute multiples of 512."""
    out = []
    while a < b:
        n = min((a // 512 + 1) * 512, b) - a
        out.append((a, a + n))
        a += n
    return out


def _build_program(dmax):
    import concourse.bacc as bacc
    import concourse.tile as tile
    import concourse.mybir as mybir

    dt = mybir.dt
    AF = mybir.ActivationFunctionType
    ALU = mybir.AluOpType
    AX = mybir.AxisListType

    PW = min(NJJ - 1, QB + dmax)

    nc = bacc.Bacc(
        "TRN2",
        target_bir_lowering=False,
        debug=False,
        enable_asserts=False,
        num_devices=NC,
    )

    kt_d = nc.dram_tensor("kt", [D, N], dt.bfloat16, kind="ExternalInput").ap()
    koh_d = nc.dram_tensor("koh", [128, NJJ * 64], dt.bfloat16, kind="ExternalInput").ap()
    klab_d = nc.dram_tensor("klab", [128, NJJ], dt.float32, kind="ExternalInput").ap()
    qlabb_d = nc.dram_tensor("qlabb", [128, RPC], dt.bfloat16, kind="ExternalInput").ap()
    qoh2_d = nc.dram_tensor("qoh2", [128, RPC], dt.float32, kind="ExternalInput").ap()

    rowacc_d = nc.dram_tensor("rowacc", [128, NJJ - 1], dt.float32, kind="ExternalOutput").ap()
    poskey_d = nc.dram_tensor("poskey", [128, PW], dt.float32, kind="ExternalOutput").ap()
    aq_d = nc.dram_tensor("aq", [1, RPC], dt.float32, kind="ExternalOutput").ap()
    pq_d = nc.dram_tensor("pq", [1, RPC], dt.float32, kind="ExternalOutput").ap()

    with tile.TileContext(nc) as tc:
        with (
            tc.tile_pool(name="keys", bufs=1) as keys_pool,
            tc.tile_pool(name="aux", bufs=1) as aux_pool,
            tc.tile_pool(name="ps", bufs=2, space="PSUM") as psum_pool,
            tc.tile_pool(name="sacc", bufs=1, space="PSUM") as sacc_pool,
            tc.tile_pool(name="ex", bufs=10) as exp_pool,
            tc.tile_pool(name="mk", bufs=5) as msk_pool,
            tc.tile_pool(name="ac", bufs=8) as acc_pool,
            tc.tile_pool(name="fin", bufs=1) as fin_pool,
        ):
            # queries (first 2048 cols) as 2 x 1024 tiles: lets sim
            # matmuls stream 1024 bf16 cols in one instruction
            kt0s = []
            for j in range(2):
                t0 = keys_pool.tile([D, 1024], dt.bfloat16, tag=f"kq{j}", name=f"kq{j}")
                nc.sync.dma_start(t0[:], kt_d[:, j * 1024:(j + 1) * 1024])
                kt0s.append(t0)
            kts = [None]
            for j in range(1, 8):
                ktile = keys_pool.tile([D, 2048], dt.bfloat16, tag=f"kt{j}", name=f"kt{j}")
                nc.sync.dma_start(ktile[:], kt_d[:, j * 2048:(j + 1) * 2048])
                kts.append(ktile)

            def kt_block(b):  # [128, 128] lhsT slice for key block b
                col = b * 128
                if col < 2048:
                    return kt0s[col // 1024][:, col % 1024:col % 1024 + 128]
                return kts[col // 2048][:, col % 2048:col % 2048 + 128]

            def kq_seg(a, b):  # rhs query slice, must sit in one 1024-tile
                assert a // 1024 == (b - 1) // 1024
                return kt0s[a // 1024][:, a % 1024:a % 1024 + (b - a)]

            koh_s = aux_pool.tile([128, NJJ * 64], dt.bfloat16, tag="koh")
            nc.sync.dma_start(koh_s[:], koh_d[:])
            klab_s = aux_pool.tile([128, NJJ], dt.float32, tag="klab")
            nc.sync.dma_start(klab_s[:], klab_d[:])
            qlabb_s = aux_pool.tile([128, RPC], dt.bfloat16, tag="qlabb")
            nc.sync.dma_start(qlabb_s[:], qlabb_d[:])
            qoh2_s = aux_pool.tile([128, RPC], dt.float32, tag="qoh2")
            nc.sync.dma_start(qoh2_s[:], qoh2_d[:])

            zoh = aux_pool.tile([128, 128], dt.bfloat16, tag="zoh")
            nc.vector.memset(zoh[:], 0.0)
            ones128 = aux_pool.tile([128, 1], dt.float32, tag="ones128")
            nc.vector.memset(ones128[:], 1.0)
            dummy = aux_pool.tile([128, 1024], dt.bfloat16, tag="dummy")

            rowacc = fin_pool.tile([128, NJJ - 1], dt.float32, tag="rowacc")
            poskey = fin_pool.tile([128, PW], dt.float32, tag="poskey")

            # doubled S accumulator: rows 0-63 and 64-127 are independent
            # halves (col-group tiling); folded by the final ones-matmul.
            # Zero-init is emitted after do_jj(0) (which has no one-hot
            # matmuls) so the first sim matmuls don't wait on the memset.
            S = sacc_pool.tile([128, RPC], dt.float32, tag="sacc")

            def init_S():
                for t in range(RPC // 512):
                    nc.tensor.matmul(
                        S[:, t * 512:(t + 1) * 512], zoh[:],
                        kt0s[t // 2][:, (t % 2) * 512:(t % 2) * 512 + 512],
                        start=True, stop=False, skip_group_check=True,
                    )

            oh_ctr = [0]

            def oh_mm(jj, a, b, stop, ex, w0):
                for (s0, s1) in _seg512(a, b):
                    th = oh_ctr[0] % 2
                    oh_ctr[0] += 1
                    nc.tensor.matmul(
                        S[th * 64:(th + 1) * 64, s0:s1],
                        koh_s[:, jj * 64:(jj + 1) * 64],
                        ex[:, s0 - w0:s1 - w0],
                        start=False, stop=stop,
                        skip_group_check=True,
                        tile_position=(0, th * 64),
                    )

            wctr = [0]
            stt_eng = nc.gpsimd if STT_ON_GPSIMD else nc.vector

            # deferred-emission queue: dependent work (oh matmuls, STT,
            # reduces) is emitted one window late so no engine queue head
            # ever waits on the exp of the window just issued
            pend = []

            def flush_pend(keep):
                while len(pend) > keep:
                    for fn in pend.pop(0):
                        fn()

            def do_jj(jj):
                main_lo = max(0, jj - 63)
                hi = min(QB - 1, jj)
                lo = (jj - 64 if jj >= 64 else main_lo) * 128
                end = (hi + 1) * 128
                acc_lo = main_lo * 128
                oh_end = min(end, jj * 128) if jj <= QB - 1 else end
                stt_lo = max(acc_lo, (jj - dmax) * 128) if jj <= QB - 1 + dmax else None

                racc = acc_pool.tile([128, 2], dt.float32, tag="racc", name="racc")
                pacc = None
                if stt_lo is not None and stt_lo < end:
                    pacc = acc_pool.tile([128, 2], dt.float32, tag="pacc", name="pacc")
                acc_k = 0
                stt_k = 0
                w0 = (lo // 1024) * 1024
                while w0 < end:
                    p_lo = max(w0, lo)
                    p_end = min(w0 + 1024, end)
                    if p_lo < p_end:
                        ps = psum_pool.tile([128, 1024], dt.float32, tag="ps", name="ps")
                        for (s0, s1) in _seg512(p_lo, p_end):
                            nc.tensor.matmul(
                                ps[:, s0 - w0:s1 - w0],
                                kt_block(jj), kq_seg(s0, s1),
                            )
                        ex = exp_pool.tile([128, 1024], dt.bfloat16, tag="ex", name="ex")
                        # extra (d=64) part: exp only, no accum
                        has_extra = p_lo < acc_lo
                        if has_extra:
                            nc.scalar.activation(
                                ex[:, p_lo - w0:acc_lo - w0],
                                ps[:, p_lo - w0:acc_lo - w0],
                                AF.Exp, scale=1.0 / TEMP,
                            )
                        a0 = max(p_lo, acc_lo)
                        if a0 < p_end:
                            use_dve = (not has_extra) and (
                                (wctr[0] * DVE_FRAC_NUM) // DVE_FRAC_DEN
                                != ((wctr[0] + 1) * DVE_FRAC_NUM) // DVE_FRAC_DEN)
                            wctr[0] += 1
                            kk = acc_k
                            if use_dve:
                                exu = ex[:, a0 - w0:p_end - w0].bitcast(dt.uint16)
                                nc.vector.tensor_scalar(
                                    exu, ps[:, a0 - w0:p_end - w0],
                                    SMULT, CADD, ALU.mult, ALU.add)
                                nc.vector.tensor_scalar(
                                    dummy[:, :p_end - a0],
                                    ex[:, a0 - w0:p_end - w0],
                                    1.0, 0.0, ALU.mult, ALU.add,
                                    accum_out=racc[:, kk:kk + 1])
                            else:
                                nc.scalar.activation(
                                    ex[:, a0 - w0:p_end - w0],
                                    ps[:, a0 - w0:p_end - w0],
                                    AF.Exp, scale=1.0 / TEMP,
                                    accum_out=racc[:, acc_k:acc_k + 1],
                                )
                            acc_k += 1
                        # deferred: one-hot class sums + positives STT for
                        # this window, emitted during the next window
                        todo = []
                        if p_lo < acc_lo:
                            todo.append(lambda jj=jj, p_lo=p_lo, acc_lo=acc_lo,
                                        ex=ex, w0=w0:
                                        oh_mm(jj, p_lo, acc_lo, True, ex, w0))
                        if a0 < min(p_end, oh_end):
                            todo.append(lambda jj=jj, a0=a0,
                                        e=min(p_end, oh_end), ex=ex, w0=w0:
                                        oh_mm(jj, a0, e, False, ex, w0))
                        if pacc is not None:
                            o0 = max(p_lo, stt_lo)
                            if o0 < p_end:
                                kk = stt_k

                                def stt(jj=jj, o0=o0, p_end=p_end, ex=ex,
                                        w0=w0, pacc=pacc, kk=kk):
                                    mk = msk_pool.tile(
                                        [128, 1024], dt.bfloat16, tag="mk", name="mk")
                                    stt_eng.scalar_tensor_tensor(
                                        mk[:, :p_end - o0], qlabb_s[:, o0:p_end],
                                        klab_s[:, jj:jj + 1],
                                        ex[:, o0 - w0:p_end - w0],
                                        ALU.is_equal, ALU.mult,
                                        accum_out=pacc[:, kk:kk + 1],
                                    )
                                todo.append(stt)
                                stt_k += 1
                        pend.append(todo)
                        flush_pend(8)
                    w0 += 1024
                fin = []
                if acc_k:
                    fin.append(lambda jj=jj, racc=racc, n=acc_k:
                               nc.vector.tensor_reduce(
                                   rowacc[:, jj:jj + 1], racc[:, :n],
                                   AX.X, ALU.add))
                if pacc is not None and stt_k:
                    fin.append(lambda jj=jj, pacc=pacc, n=stt_k:
                               nc.vector.tensor_reduce(
                                   poskey[:, jj:jj + 1], pacc[:, :n],
                                   AX.X, ALU.add))
                if fin:
                    pend.append(fin)

            sc = fin_pool.tile([128, RPC], dt.float32, tag="sc")
            pm = fin_pool.tile([128, RPC], dt.float32, tag="pm")

            do_jj(0)
            init_S()
            # interleave small head/tail jj's among full-width ones so the
            # pipeline never runs a long stretch of short windows
            fulls = list(range(16, 64))
            others = list(range(1, 16)) + list(range(64, NJJ))
            seq = []
            fi = oi = 0
            while fi < len(fulls) or oi < len(others):
                if oi >= len(others) or (
                        fi < len(fulls)
                        and fi * len(others) <= oi * len(fulls)):
                    seq.append(fulls[fi]); fi += 1
                else:
                    seq.append(others[oi]); oi += 1

            def oh_iv(jj):  # cols covered by jj's oh matmuls
                main_lo = max(0, jj - 63)
                hi = min(QB - 1, jj)
                lo = (jj - 64 if jj >= 64 else main_lo) * 128
                end = (hi + 1) * 128
                oh_end = min(end, jj * 128) if jj <= QB - 1 else end
                return lo, oh_end

            last_toucher = {}
            for b in range(4):
                c0, c1 = b * 512, (b + 1) * 512
                for pos, jj in enumerate(seq):
                    lo, oh_end = oh_iv(jj)
                    if lo < c1 and oh_end > c0:
                        last_toucher[b] = pos

            def drain(b):
                nc.scalar.copy(
                    sc[:, b * 512:(b + 1) * 512],
                    S[:, b * 512:(b + 1) * 512])
                nc.vector.tensor_mul(
                    pm[:, b * 512:(b + 1) * 512],
                    sc[:, b * 512:(b + 1) * 512],
                    qoh2_s[:, b * 512:(b + 1) * 512])

            for pos, jj in enumerate(seq):
                do_jj(jj)
                for b in range(4):
                    if last_toucher.get(b) == pos:
                        pend.append([lambda b=b: drain(b)])
            flush_pend(0)

            # tail: fold halves via ones-matmuls
            paq = sacc_pool.tile([1, RPC], dt.float32, tag="sacc", name="paq")
            for t in range(RPC // 512):
                nc.tensor.matmul(
                    paq[:, t * 512:(t + 1) * 512], ones128[:],
                    sc[:, t * 512:(t + 1) * 512],
                )
            aq_s = fin_pool.tile([1, RPC], dt.float32, tag="aqs")
            nc.vector.tensor_copy(aq_s[:], paq[:])
            nc.sync.dma_start(aq_d[:], aq_s[:])

            pq_s = fin_pool.tile([1, RPC], dt.float32, tag="pqs")
            for h in range(2):
                ppq = psum_pool.tile([1, 1024], dt.float32, tag="ps", name="ppq")
                for t in range(2):
                    nc.tensor.matmul(
                        ppq[:, t * 512:(t + 1) * 512], ones128[:],
                        pm[:, h * 1024 + t * 512:h * 1024 + (t + 1) * 512],
                    )
                nc.vector.tensor_copy(pq_s[:, h * 1024:(h + 1) * 1024], ppq[:])
            nc.sync.dma_start(pq_d[:], pq_s[:])

            nc.sync.dma_start(rowacc_d[:], rowacc[:])
            nc.sync.dma_start(poskey_d[:], poskey[:])

    nc.compile()
    return nc, PW


def _compute_dmax(lab_s):
    first = lab_s.reshape(NBLK, 128)[:, 0]
    last = lab_s.reshape(NBLK, 128)[:, -1]
    dmax = 0
    for jj in range(NBLK):
        i = jj
        while i > 0 and last[i - 1] >= first[jj]:
            i -= 1
        dmax = max(dmax, jj - i)
    return max(1, min(dmax, 63))


def get_program(dmax):
    key = ("v3", dmax)
    if key not in _prog_cache:
        _prog_cache[key] = _build_program(dmax)
    return _prog_cache[key]


def make_in_maps(embeddings, partition_labels):
    emb = np.asarray(embeddings, dtype=np.float32)
    labels = np.asarray(partition_labels).astype(np.int64)
    perm = np.argsort(labels, kind="stable")
    E_s = emb[perm]
    lab_s = labels[perm]
    lab_f = lab_s.astype(np.float32)

    dmax = _compute_dmax(lab_s)
    E_sT = np.ascontiguousarray(E_s.T).astype(BF16)
    dia = np.exp(np.sum(E_s.astype(np.float64) ** 2, axis=1) / TEMP)

    cls = np.arange(64, dtype=np.int64)
    in_maps = []
    for c in range(NC):
        idx = (np.arange(N) + c * RPC) % N
        ktrot = np.ascontiguousarray(E_sT[:, idx])
        kl = lab_f[idx[:NJJ * 128]].reshape(NJJ, 128).T
        koh = (lab_s[idx[:NJJ * 128]].reshape(NJJ, 128)[:, :, None]
               == cls[None, None, :])
        koh = np.ascontiguousarray(
            koh.transpose(1, 0, 2).reshape(128, NJJ * 64)).astype(BF16)
        qlab_c = lab_f[c * RPC:(c + 1) * RPC]
        qlabb = np.ascontiguousarray(
            np.broadcast_to(qlab_c.astype(BF16)[None, :], (128, RPC)))
        qoh = (lab_s[c * RPC:(c + 1) * RPC][None, :] == cls[:, None])
        qoh2 = np.ascontiguousarray(
            np.vstack([qoh, qoh])).astype(np.float32)      # [128, RPC]
        in_maps.append({
            "kt": ktrot,
            "koh": koh,
            "klab": np.ascontiguousarray(kl).astype(np.float32),
            "qlabb": qlabb,
            "qoh2": qoh2,
        })
    return in_maps, lab_s, dmax, dia


def combine(results, lab_s, PW, dia):
    A = np.zeros(N, dtype=np.float64)
    P = np.zeros(N, dtype=np.float64)
    for c, r in enumerate(results):
        base = c * RPC
        A[base:base + RPC] += np.asarray(r["aq"], dtype=np.float64)[0]
        P[base:base + RPC] += np.asarray(r["pq"], dtype=np.float64)[0]
        ra = np.asarray(r["rowacc"], dtype=np.float64)
        pk = np.asarray(r["poskey"], dtype=np.float64)
        for jj in range(NJJ - 1):
            g = (base + jj * 128) % N
            A[g:g + 128] += ra[:, jj]
            if jj < PW:
                P[g:g + 128] += pk[:, jj]
    A -= dia
    P -= dia

    counts = np.bincount(lab_s, minlength=1)
    valid = counts[lab_s] >= 2
    n_valid = int(valid.sum())
    if n_valid == 0:
        return np.float32(0.0)
    loss = np.log(A) - np.log(np.maximum(P, 1e-300))
    return np.float32(loss[valid].sum() / n_valid)


def kernel(embeddings, partition_labels):
    from concourse.bass_utils import run_bass_kernel_spmd

    in_maps, lab_s, dmax, dia = make_in_maps(embeddings, partition_labels)
    nc, PW = get_program(dmax)
    res = run_bass_kernel_spmd(nc, in_maps, list(range(NC)))
    return combine(res.results, lab_s, PW, dia)

